# revision 25
# baseline (speedup 1.0000x reference)
"""Trainium2 Bass kernel for the EntityRelationHead problem.

Shapes (hardcoded): h [2,512,768], proj_W [768,768], proj_b [768],
ent_W [9,768], ent_b [9], bil_W [32,768,768], bil_b [32].
Outputs: ent_logits [2,512,9], rel_logits [2,512,512,32].

Sharding: 8 cores = 2 batches x 4 relation-groups (8 relations each).
Each core computes hpT = relu(W_p @ h_b^T + b) for its batch, then for each
of its 8 relations r: tT_r = W_r^T @ hpT and rel_r = tT_r^T @ hpT + b_r.
All matmuls run in float32r (single-pass fp32, full PE rate at N=512).
"""

import numpy as np

import concourse.bass as bass  # noqa: F401  (bass types referenced via tile/bacc)
import concourse.mybir as mybir
import concourse.tile as tile
from concourse import bacc
from concourse.bass_utils import run_bass_kernel_spmd

F32 = mybir.dt.float32
F32R = mybir.dt.float32r
AF = mybir.ActivationFunctionType

B, L, H, E, R = 2, 512, 768, 9, 32
NCORES = 8
GROUPS = NCORES // B  # 4 relation groups per batch
RPC = R // GROUPS  # 8 relations per core
P = 128
KT = H // P  # 6 contraction tiles
IT = L // P  # 4 output row tiles for rel

_CACHE = {}


def _build():
    nc = bacc.Bacc(None, target_bir_lowering=False)

    hT_d = nc.dram_tensor("hT", [H, L], F32R, kind="ExternalInput")
    projWT_d = nc.dram_tensor("projWT", [H, H], F32R, kind="ExternalInput")
    projb_d = nc.dram_tensor("projb", [P, KT], F32, kind="ExternalInput")
    entWT_d = nc.dram_tensor("entWT", [H, E], F32R, kind="ExternalInput")
    entb_d = nc.dram_tensor("entb", [E, 1], F32, kind="ExternalInput")
    bilW_d = nc.dram_tensor("bilW", [RPC, H, H], F32R, kind="ExternalInput")
    bilb_d = nc.dram_tensor("bilb", [P, RPC], F32, kind="ExternalInput")

    entT_o = nc.dram_tensor("entT", [E, L], F32, kind="ExternalOutput")
    rel_o = nc.dram_tensor("rel", [RPC, L, L], F32, kind="ExternalOutput")

    hT_r = hT_d.rearrange("(t p) l -> p t l", p=P)
    entWT_r = entWT_d.rearrange("(t p) e -> p t e", p=P)

    with tile.TileContext(nc) as tc:
        with (
            tc.tile_pool(name="const", bufs=1) as const,
            tc.tile_pool(name="wpool", bufs=2) as wpool,
            tc.tile_pool(name="tpool", bufs=2) as tpool,
            tc.tile_pool(name="opool", bufs=6) as opool,
            tc.tile_pool(name="psum", bufs=7, space="PSUM") as psum,
        ):
            hT = const.tile([P, KT, L], F32R)
            projWT = const.tile([P, KT, KT, P], F32R)  # [p, k, m, col]
            projb = const.tile([P, KT], F32)
            entWT = const.tile([P, KT, E], F32R)
            entb = const.tile([E, 1], F32)
            bilb = const.tile([P, RPC], F32)
            hpT = const.tile([P, KT, L], F32R)
            entT = const.tile([E, L], F32)
            warm = const.tile([P, L], mybir.dt.bfloat16)

            nc.gpsimd.memset(warm[:], 0.0)
            # phase-1-critical inputs first: hT by k-chunk (scalar ring),
            # projWT by m-column-block (sync ring) so the m-outer proj loop
            # can start after hT + one column block instead of everything
            for k in range(KT):
                nc.scalar.dma_start(hT[:, k, :], hT_r[:, k, :])
            for m in range(KT):
                nc.sync.dma_start(
                    projWT[:, :, m, :],
                    projWT_d[:, m * P : (m + 1) * P].rearrange(
                        "(t p) c -> p t c", p=P
                    ),
                )
            nc.scalar.dma_start(projb[:], projb_d[:])
            nc.scalar.dma_start(entb[:], entb_d[:])
            nc.scalar.dma_start(bilb[:], bilb_d[:])
            nc.scalar.dma_start(entWT[:], entWT_r)

            # ---- phase 0: PE warm-up ----
            # dummy matmuls keep the PE busy during the input-DMA head so the
            # HAM clock gate reaches 8/8 before the first real matmul.
            eps = psum.tile([E, L], F32, tag="ent", bufs=1)

            # ---- phase 1: hpT[o, l] = relu(projWT.T @ hT + projb) ----
            # m-outer: group m needs all of hT but only column block m of
            # projWT, so the first group starts after ~1.9 MB instead of 3.9
            pps = [
                psum.tile([P, L], F32, tag="mm", name=f"pps{m}") for m in range(KT)
            ]
            for w in range(8):
                nc.tensor.matmul(
                    pps[0][:], warm[:, :P], warm[:], start=True, stop=True
                )
            for m in range(KT):
                for k in range(KT):
                    nc.tensor.matmul(
                        pps[m][:],
                        projWT[:, k, m, :],
                        hT[:, k, :],
                        start=(k == 0),
                        stop=(k == KT - 1),
                    )
                # evictions alternate ACT/DVE and overlap the next m group
                if m % 2 == 0:
                    nc.scalar.activation(
                        hpT[:, m, :], pps[m][:], AF.Relu, bias=projb[:, m : m + 1]
                    )
                else:
                    nc.vector.tensor_scalar(
                        hpT[:, m, :],
                        pps[m][:],
                        scalar1=projb[:, m : m + 1],
                        scalar2=0.0,
                        op0=mybir.AluOpType.add,
                        op1=mybir.AluOpType.max,
                    )

            # ---- phase 2: entT[e, l] = entWT.T @ hpT + entb ----
            for k in range(KT):
                nc.tensor.matmul(
                    eps[:],
                    entWT[:, k, :],
                    hpT[:, k, :],
                    start=(k == 0),
                    stop=(k == KT - 1),
                )
            nc.scalar.activation(entT[:], eps[:], AF.Identity, bias=entb[:])
            nc.sync.dma_start(entT_o[:], entT[:])

            # ---- phase 3: per relation r, software-pipelined ----
            # Emit tT(r+1) before rel(r): the PE's static stream then never
            # waits on the PSUM->SBUF evictions of tT(r) (they complete while
            # tT(r+1) matmuls run).
            def emit_tT(r):
                bilW = wpool.tile([P, KT, H], F32R, tag="bilW", name=f"bilW{r}")
                bilW_r = bilW_d[r].rearrange("(t p) k -> p t k", p=P)
                # per-k chunk DMAs + k-outer matmuls: the first round only
                # needs chunk 0, so tT starts ~6x earlier after a DMA stall
                for k in range(KT):
                    nc.sync.dma_start(bilW[:, k, :], bilW_r[:, k, :])
                tT = tpool.tile([P, KT, L], F32R, tag="tT", name=f"tT{r}")
                tps = [
                    psum.tile([P, L], F32, tag="mm", name=f"t_ps{r}_{m}")
                    for m in range(KT)
                ]
                for k in range(KT):
                    for m in range(KT):
                        nc.tensor.matmul(
                            tps[m][:],
                            bilW[:, k, m * P : (m + 1) * P],
                            hpT[:, k, :],
                            start=(k == 0),
                            stop=(k == KT - 1),
                        )
                for m in range(KT):
                    nc.vector.tensor_copy(tT[:, m, :], tps[m][:])
                return tT

            def emit_rel(r, tT):
                for i in range(IT):
                    ps = psum.tile([P, L], F32, tag="mm", name=f"r_ps{r}_{i}")
                    for k in range(KT):
                        nc.tensor.matmul(
                            ps[:],
                            tT[:, k, i * P : (i + 1) * P],
                            hpT[:, k, :],
                            start=(k == 0),
                            stop=(k == KT - 1),
                        )
                    out = opool.tile([P, L], F32, tag="rel", name=f"rel{r}_{i}")
                    if i % 2 == 0:
                        nc.scalar.activation(
                            out[:], ps[:], AF.Identity, bias=bilb[:, r : r + 1]
                        )
                    else:
                        nc.vector.tensor_scalar(
                            out[:],
                            ps[:],
                            scalar1=bilb[:, r : r + 1],
                            scalar2=None,
                            op0=mybir.AluOpType.add,
                        )
                    nc.sync.dma_start(rel_o[r, i * P : (i + 1) * P, :], out[:])

            tT_cur = emit_tT(0)
            for r in range(RPC):
                tT_next = emit_tT(r + 1) if r + 1 < RPC else None
                emit_rel(r, tT_cur)
                tT_cur = tT_next

    nc.compile()
    return nc


def _get_nc():
    if "nc" not in _CACHE:
        _CACHE["nc"] = _build()
    return _CACHE["nc"]


def kernel(h, proj_W, proj_b, ent_W, ent_b, bil_W, bil_b, **kwargs):
    h = np.asarray(h, dtype=np.float32)
    proj_W = np.asarray(proj_W, dtype=np.float32)
    proj_b = np.asarray(proj_b, dtype=np.float32)
    ent_W = np.asarray(ent_W, dtype=np.float32)
    ent_b = np.asarray(ent_b, dtype=np.float32)
    bil_W = np.asarray(bil_W, dtype=np.float32)
    bil_b = np.asarray(bil_b, dtype=np.float32)

    nc = _get_nc()

    projWT = np.ascontiguousarray(proj_W.T)
    projb = np.ascontiguousarray(proj_b.reshape(KT, P).T)
    entWT = np.ascontiguousarray(ent_W.T)
    entb = np.ascontiguousarray(ent_b.reshape(E, 1))

    in_maps = []
    for c in range(NCORES):
        b = c // GROUPS
        g = c % GROUPS
        rsl = slice(g * RPC, (g + 1) * RPC)
        in_maps.append(
            {
                "hT": np.ascontiguousarray(h[b].T),
                "projWT": projWT,
                "projb": projb,
                "entWT": entWT,
                "entb": entb,
                "bilW": np.ascontiguousarray(bil_W[rsl]),
                "bilb": np.ascontiguousarray(
                    np.broadcast_to(bil_b[rsl], (P, RPC))
                ),
            }
        )

    res = run_bass_kernel_spmd(nc, in_maps, core_ids=list(range(NCORES)))
    results = res.results

    ent_logits = np.empty((B, L, E), dtype=np.float32)
    rel_logits = np.empty((B, L, L, R), dtype=np.float32)
    for c in range(NCORES):
        b = c // GROUPS
        g = c % GROUPS
        rsl = slice(g * RPC, (g + 1) * RPC)
        if g == 0:
            ent_logits[b] = results[c]["entT"].T
        rel_logits[b][:, :, rsl] = results[c]["rel"].transpose(1, 2, 0)
    return ent_logits, rel_logits


# revision 28
# speedup vs baseline: 1.0234x; 1.0234x over previous
"""Trainium2 Bass kernel for the EntityRelationHead problem.

Shapes (hardcoded): h [2,512,768], proj_W [768,768], proj_b [768],
ent_W [9,768], ent_b [9], bil_W [32,768,768], bil_b [32].
Outputs: ent_logits [2,512,9], rel_logits [2,512,512,32].

Sharding: 8 cores = 2 batches x 4 relation-groups (8 relations each).
Each core computes hpT = relu(W_p @ h_b^T + b) for its batch, then for each
of its 8 relations r: tT_r = W_r^T @ hpT and rel_r = tT_r^T @ hpT + b_r.
All matmuls run in float32r (single-pass fp32, full PE rate at N=512).
"""

import numpy as np

import concourse.bass as bass  # noqa: F401  (bass types referenced via tile/bacc)
import concourse.mybir as mybir
import concourse.tile as tile
from concourse import bacc
from concourse.bass_utils import run_bass_kernel_spmd

F32 = mybir.dt.float32
F32R = mybir.dt.float32r
AF = mybir.ActivationFunctionType

B, L, H, E, R = 2, 512, 768, 9, 32
NCORES = 8
GROUPS = NCORES // B  # 4 relation groups per batch
RPC = R // GROUPS  # 8 relations per core
P = 128
KT = H // P  # 6 contraction tiles
IT = L // P  # 4 output row tiles for rel

_CACHE = {}


def _build():
    nc = bacc.Bacc(None, target_bir_lowering=False)

    hT_d = nc.dram_tensor("hT", [H, L], F32R, kind="ExternalInput")
    projWT_d = nc.dram_tensor("projWT", [H, H], F32R, kind="ExternalInput")
    projb_d = nc.dram_tensor("projb", [P, KT], F32, kind="ExternalInput")
    entWT_d = nc.dram_tensor("entWT", [H, E], F32R, kind="ExternalInput")
    entb_d = nc.dram_tensor("entb", [E, 1], F32, kind="ExternalInput")
    bilW_d = nc.dram_tensor("bilW", [RPC, H, H], F32R, kind="ExternalInput")
    bilb_d = nc.dram_tensor("bilb", [P, RPC], F32, kind="ExternalInput")

    entT_o = nc.dram_tensor("entT", [E, L], F32, kind="ExternalOutput")
    rel_o = nc.dram_tensor("rel", [RPC, L, L], F32, kind="ExternalOutput")

    hT_r = hT_d.rearrange("(t p) l -> p t l", p=P)
    entWT_r = entWT_d.rearrange("(t p) e -> p t e", p=P)

    with tile.TileContext(nc) as tc:
        with (
            tc.tile_pool(name="const", bufs=1) as const,
            tc.tile_pool(name="wpool", bufs=2) as wpool,
            tc.tile_pool(name="tpool", bufs=2) as tpool,
            tc.tile_pool(name="opool", bufs=6) as opool,
            tc.tile_pool(name="psum", bufs=7, space="PSUM") as psum,
        ):
            hT = const.tile([P, KT, L], F32R)
            projWT = const.tile([P, KT, H], F32R)
            projb = const.tile([P, KT], F32)
            entWT = const.tile([P, KT, E], F32R)
            entb = const.tile([E, 1], F32)
            bilb = const.tile([P, RPC], F32)
            hpT = const.tile([P, KT, L], F32R)
            entT = const.tile([E, L], F32)
            warm = const.tile([P, L], mybir.dt.bfloat16)

            nc.gpsimd.memset(warm[:], 0.0)
            # phase-1-critical chunks first, per-k so the k-outer proj loop
            # starts as soon as the first chunks land; hT and projWT ride
            # different HWDGE rings so their issue costs overlap
            projWT_r = projWT_d.rearrange("(t p) o -> p t o", p=P)
            for k in range(KT):
                nc.scalar.dma_start(hT[:, k, :], hT_r[:, k, :])
                nc.sync.dma_start(projWT[:, k, :], projWT_r[:, k, :])
            nc.scalar.dma_start(projb[:], projb_d[:])
            nc.scalar.dma_start(entb[:], entb_d[:])
            nc.scalar.dma_start(bilb[:], bilb_d[:])
            nc.scalar.dma_start(entWT[:], entWT_r)

            # ---- phase 0: PE warm-up ----
            # dummy matmuls keep the PE busy during the input-DMA head so the
            # HAM clock gate reaches 8/8 before the first real matmul.
            eps = psum.tile([E, L], F32, tag="ent", bufs=1)

            # ---- phase 1: hpT[o, l] = relu(projWT.T @ hT + projb) ----
            # k-outer so matmuls start as soon as the first k-chunks land
            pps = [
                psum.tile([P, L], F32, tag="mm", name=f"pps{m}") for m in range(KT)
            ]
            for w in range(8):
                nc.tensor.matmul(
                    pps[0][:], warm[:, :P], warm[:], start=True, stop=True
                )
            for k in range(KT):
                for m in range(KT):
                    nc.tensor.matmul(
                        pps[m][:],
                        projWT[:, k, m * P : (m + 1) * P],
                        hT[:, k, :],
                        start=(k == 0),
                        stop=(k == KT - 1),
                    )
            # evictions alternate ACT/DVE so the tail clears ~2x faster
            for m in range(KT):
                if m % 2 == 0:
                    nc.scalar.activation(
                        hpT[:, m, :], pps[m][:], AF.Relu, bias=projb[:, m : m + 1]
                    )
                else:
                    nc.vector.tensor_scalar(
                        hpT[:, m, :],
                        pps[m][:],
                        scalar1=projb[:, m : m + 1],
                        scalar2=0.0,
                        op0=mybir.AluOpType.add,
                        op1=mybir.AluOpType.max,
                    )

            # ---- phase 2: entT[e, l] = entWT.T @ hpT + entb ----
            for k in range(KT):
                nc.tensor.matmul(
                    eps[:],
                    entWT[:, k, :],
                    hpT[:, k, :],
                    start=(k == 0),
                    stop=(k == KT - 1),
                )
            nc.scalar.activation(entT[:], eps[:], AF.Identity, bias=entb[:])
            nc.sync.dma_start(entT_o[:], entT[:])

            # ---- phase 3: per relation r, software-pipelined ----
            # Emit tT(r+1) before rel(r): the PE's static stream then never
            # waits on the PSUM->SBUF evictions of tT(r) (they complete while
            # tT(r+1) matmuls run).
            def emit_tT(r):
                bilW = wpool.tile([P, KT, H], F32R, tag="bilW", name=f"bilW{r}")
                bilW_r = bilW_d[r].rearrange("(t p) k -> p t k", p=P)
                # per-k chunk DMAs + k-outer matmuls: the first round only
                # needs chunk 0, so tT starts ~6x earlier after a DMA stall
                for k in range(KT):
                    nc.sync.dma_start(bilW[:, k, :], bilW_r[:, k, :])
                tT = tpool.tile([P, KT, L], F32R, tag="tT", name=f"tT{r}")
                tps = [
                    psum.tile([P, L], F32, tag="mm", name=f"t_ps{r}_{m}")
                    for m in range(KT)
                ]
                for k in range(KT):
                    for m in range(KT):
                        nc.tensor.matmul(
                            tps[m][:],
                            bilW[:, k, m * P : (m + 1) * P],
                            hpT[:, k, :],
                            start=(k == 0),
                            stop=(k == KT - 1),
                        )
                for m in range(KT):
                    nc.vector.tensor_copy(tT[:, m, :], tps[m][:])
                return tT

            def emit_rel(r, tT):
                for i in range(IT):
                    ps = psum.tile([P, L], F32, tag="mm", name=f"r_ps{r}_{i}")
                    for k in range(KT):
                        nc.tensor.matmul(
                            ps[:],
                            tT[:, k, i * P : (i + 1) * P],
                            hpT[:, k, :],
                            start=(k == 0),
                            stop=(k == KT - 1),
                        )
                    out = opool.tile([P, L], F32, tag="rel", name=f"rel{r}_{i}")
                    if i % 2 == 0:
                        nc.scalar.activation(
                            out[:], ps[:], AF.Identity, bias=bilb[:, r : r + 1]
                        )
                    else:
                        nc.vector.tensor_scalar(
                            out[:],
                            ps[:],
                            scalar1=bilb[:, r : r + 1],
                            scalar2=None,
                            op0=mybir.AluOpType.add,
                        )
                    nc.sync.dma_start(rel_o[r, i * P : (i + 1) * P, :], out[:])

            tT_cur = emit_tT(0)
            for r in range(RPC):
                tT_next = emit_tT(r + 1) if r + 1 < RPC else None
                emit_rel(r, tT_cur)
                tT_cur = tT_next

    nc.compile()
    return nc


def _get_nc():
    if "nc" not in _CACHE:
        _CACHE["nc"] = _build()
    return _CACHE["nc"]


def kernel(h, proj_W, proj_b, ent_W, ent_b, bil_W, bil_b, **kwargs):
    h = np.asarray(h, dtype=np.float32)
    proj_W = np.asarray(proj_W, dtype=np.float32)
    proj_b = np.asarray(proj_b, dtype=np.float32)
    ent_W = np.asarray(ent_W, dtype=np.float32)
    ent_b = np.asarray(ent_b, dtype=np.float32)
    bil_W = np.asarray(bil_W, dtype=np.float32)
    bil_b = np.asarray(bil_b, dtype=np.float32)

    nc = _get_nc()

    projWT = np.ascontiguousarray(proj_W.T)
    projb = np.ascontiguousarray(proj_b.reshape(KT, P).T)
    entWT = np.ascontiguousarray(ent_W.T)
    entb = np.ascontiguousarray(ent_b.reshape(E, 1))

    in_maps = []
    for c in range(NCORES):
        b = c // GROUPS
        g = c % GROUPS
        rsl = slice(g * RPC, (g + 1) * RPC)
        in_maps.append(
            {
                "hT": np.ascontiguousarray(h[b].T),
                "projWT": projWT,
                "projb": projb,
                "entWT": entWT,
                "entb": entb,
                "bilW": np.ascontiguousarray(bil_W[rsl]),
                "bilb": np.ascontiguousarray(
                    np.broadcast_to(bil_b[rsl], (P, RPC))
                ),
            }
        )

    res = run_bass_kernel_spmd(nc, in_maps, core_ids=list(range(NCORES)))
    results = res.results

    ent_logits = np.empty((B, L, E), dtype=np.float32)
    rel_logits = np.empty((B, L, L, R), dtype=np.float32)
    for c in range(NCORES):
        b = c // GROUPS
        g = c % GROUPS
        rsl = slice(g * RPC, (g + 1) * RPC)
        if g == 0:
            ent_logits[b] = results[c]["entT"].T
        rel_logits[b][:, :, rsl] = results[c]["rel"].transpose(1, 2, 0)
    return ent_logits, rel_logits
